# revision 93
# baseline (speedup 1.0000x reference)
"""Multi-head causal attention on 8 TRN2 NeuronCores.

Sharding: tensor-parallel over heads — 16 heads / 8 cores = 2 heads per core.
Each core computes q/k/v projections for its 2 heads (column-sharded QKV
weights), causal attention for those heads over both batch elements, and the
row-sharded slice of the output projection, producing a full-shape partial
output.  Host sums the 8 partials and adds bo + bv @ Wo.T (the per-head value
bias commutes through the output projection because attention rows sum to 1).

Structure (vs the fp32r baseline, 371.7us -> 269.3us cost-model time):
  - QKV projections run as fp8-e4m3 DoubleRow matmuls (0.5 cycles/row,
    two 128-deep chunks per pass).  x and the QKV weights are host-split
    into hi+lo fp8 pairs; three passes (Wh*xh + Wh*xl + Wl*xh) reproduce
    the full product of the quantized operands to ~1e-3.  Weights are
    host-scaled by 64 to dodge fp8 subnormals; the scale folds into the
    PSUM-evacuation activation.  x / weights are host-pre-tiled to the
    SBUF layout so every DMA row is >=512B (smaller rows pay 2x) and each
    array loads as one DMA (per-transfer overheads beat chunked overlap).
  - attention q/k/v/e/oT are fp16 (1 cycle/row at any width); PSUM fp32.
  - softmax denominators are NOT ones-row matmuls (a full PE pass per
    e-tile).  Per 128-wide q-chunk a transposed tiny matmul (lhsT =
    e-chunk, rhs = ones column, output [128,1], ap_size 1 ~ free)
    accumulates Z in PSUM across k-tiles in q-partition layout; 1/Z
    returns to row layout via 4 more tiny matmuls (rc_colT @ I), one
    partition-0 broadcast, and a DVE multiply.  PSUM start/stop are
    bank-granular: exactly one start (first write) and one stop (last
    write) per bank accumulation window.
  - scores are computed transposed [k, q]; diagonal k-tiles compute only
    the causally valid column range, and the causal mask bias is
    accumulated into the scores bank by the PE itself (tbT^T @ I).
  - engines execute their streams in order, so overlap is an emission-
    order property: batch-1 projections interleave into batch-0's
    attention loop; av/Z matmuls lag scores by 3 k-tiles (PE wait-queue
    depth is 4); softmax chain tails are deferred behind later PE work;
    batch-1 out-projections defer one q-tile; the last one rotates its
    PSUM over freed scores banks and spreads evacuations evenly.
  - out-projection evacuations split DVE/ACT (1-in-3; 1-in-5 in the
    saturated batch-1 phase); paired et-chunks share one osb tile and one
    output DMA to halve SP dispatch work.
"""

import sys

if "/opt/trn_rl_repo" not in sys.path:
    sys.path.insert(0, "/opt/trn_rl_repo")

import numpy as np

import concourse.bass as bass  # noqa: F401  (engine namespaces live on nc)
import concourse.tile as tile
from concourse import bacc, mybir
from concourse.bass_utils import run_bass_kernel_spmd

F32 = mybir.dt.float32
F32R = mybir.dt.float32r
F16 = mybir.dt.float16
F8 = mybir.dt.float8e4
DR = mybir.MatmulPerfMode.DoubleRow
AF = mybir.ActivationFunctionType
ALU = mybir.AluOpType

B, S, E = 2, 2048, 2048
H, D = 16, 128
NCORES = 8
HPC = H // NCORES          # heads per core = 2
M = HPC * D                # local channels per core = 256
EO = E // 128              # 16 contraction chunks
XT = 256                   # token-tile width for projections
NT = S // XT               # 8 token tiles per batch
QT = 512                   # q-tile width for attention
NQT = S // QT              # 4 q-tiles
ET = 512                   # e-tile width for out-projection
SCALE = 1.0 / float(np.sqrt(D))
MASK_BIAS = -30.0
WS = 64.0


def build_nc():
    nc = bacc.Bacc(trn_type="TRN2", target_bir_lowering=False, num_swdge_queues=4)

    xT = {s: nc.declare_dram_parameter(f"x{s}", [B, NT, 128, EO, XT], F8,
                                       isOutput=False)
          for s in "hl"}
    wqkv = {f"{n}{s}": nc.declare_dram_parameter(f"w{n}{s}", [128, EO, M], F8,
                                                 isOutput=False)
            for n in "qkv" for s in "hl"}
    wo = nc.declare_dram_parameter("wo", [M, E], F16, isOutput=False)
    bq = nc.declare_dram_parameter("bq", [128, HPC], F32, isOutput=False)
    bk = nc.declare_dram_parameter("bk", [128, HPC], F32, isOutput=False)
    tb = nc.declare_dram_parameter("tb", [128, 128], F16, isOutput=False)
    ident = nc.declare_dram_parameter("ident", [128, 128], F16, isOutput=False)
    o = nc.declare_dram_parameter("o", [B, S, E], F16, isOutput=True)

    with tile.TileContext(nc) as tc:
        _body(tc, nc, xT, wqkv, wo, bq, bk, tb, ident, o)
    nc.compile()
    return nc


def _body(tc, nc, xT, wqkv, wo, bq, bk, tb, ident, o):
    from contextlib import ExitStack

    ctx = ExitStack()
    with ctx:
        wpool = ctx.enter_context(tc.tile_pool(name="w", bufs=1))
        xpool = ctx.enter_context(tc.tile_pool(name="x", bufs=4))
        qkv = ctx.enter_context(tc.tile_pool(name="qkv", bufs=2))
        epool = ctx.enter_context(tc.tile_pool(name="e", bufs=8))
        otp = ctx.enter_context(tc.tile_pool(name="ot", bufs=2))
        osp = ctx.enter_context(tc.tile_pool(name="os", bufs=12))
        rp = ctx.enter_context(tc.tile_pool(name="r", bufs=3))
        psA = ctx.enter_context(tc.tile_pool(name="psA", bufs=2, space="PSUM"))
        psC = ctx.enter_context(tc.tile_pool(name="psC", bufs=3, space="PSUM"))
        psB = ctx.enter_context(tc.tile_pool(name="psB", bufs=2, space="PSUM"))
        psZ = ctx.enter_context(tc.tile_pool(name="psZ", bufs=1, space="PSUM"))

        # ---- weights / constants (once) ----
        wsb = {n: wpool.tile([128, EO, M], F8, tag=f"w{n}", name=f"w{n}_sb")
               for n in ("qh", "ql", "kh", "kl", "vh", "vl")}
        wo_sb = wpool.tile([128, HPC, E], F16, tag="wo")
        on_sb = wpool.tile([128, 1], F16, tag="on")
        nc.vector.memset(on_sb[:], 1.0)
        # Warm the PE (p-state ramp) with matmuls while x0/weights load.
        warm_rhs = rp.tile([128, 512], F16, tag="wr")
        nc.vector.memset(warm_rhs[:], 0.0)
        warm = psC.tile([128, 512], F32, tag="sc")
        for _ in range(18):
            nc.tensor.matmul(warm[:1, :], on_sb[:], warm_rhs[:],
                             start=True, stop=True)
        # x tile 0 and wq stream in interleaved chunks so the first projection
        # matmuls start as soon as their first contraction chunks land; wo is
        # not needed until attention output, so it loads last.
        x_first = {s: xpool.tile([128, EO, XT], F8, tag=f"x{s}", name=f"xf{s}")
                   for s in "hl"}
        nc.sync.dma_start(x_first["h"][:], xT["h"][0, 0])
        nc.scalar.dma_start(wsb["qh"][:], wqkv["qh"][:])
        nc.sync.dma_start(x_first["l"][:], xT["l"][0, 0])
        nc.scalar.dma_start(wsb["ql"][:], wqkv["ql"][:])
        nc.scalar.dma_start(wsb["kh"][:], wqkv["kh"][:])
        nc.scalar.dma_start(wsb["kl"][:], wqkv["kl"][:])
        bq_sb = wpool.tile([128, HPC], F32, tag="bq")
        bk_sb = wpool.tile([128, HPC], F32, tag="bk")
        nc.sync.dma_start(bq_sb[:], bq[:])
        nc.sync.dma_start(bk_sb[:], bk[:])
        nc.scalar.dma_start(wsb["vh"][:], wqkv["vh"][:])
        nc.scalar.dma_start(wsb["vl"][:], wqkv["vl"][:])
        x_second = {s: xpool.tile([128, EO, XT], F8, tag=f"x{s}", name=f"xs{s}")
                    for s in "hl"}
        for s in "hl":
            nc.sync.dma_start(x_second[s][:], xT[s][0, 1])
        tbT_sb = wpool.tile([128, 128], F16, tag="tb")
        nc.sync.dma_start(tbT_sb[:], tb[:])
        id_sb = wpool.tile([128, 128], F16, tag="id")
        nc.sync.dma_start(id_sb[:], ident[:])

        # per-batch state (filled by emit_proj_*, read by attention)
        st = {}

        def new_batch(b):
            qT_sb = qkv.tile([128, HPC, S], F16, tag="qT")
            kT_sb = qkv.tile([128, HPC, S], F16, tag="kT")
            v_sb = qkv.tile([128, S // 128, M], F16, tag="v")
            oT_sb = otp.tile([128, HPC, S], F16, tag="oT")
            st[b] = dict(
                qT_h=[qT_sb[:, h] for h in range(HPC)],
                kT_h=[kT_sb[:, h] for h in range(HPC)],
                v_sb=v_sb,
                v_h=[v_sb[:, :, h * D:(h + 1) * D] for h in range(HPC)],
                oT_h=[oT_sb[:, h] for h in range(HPC)],
            )

        EP = EO // 2   # DoubleRow packs two 128-deep chunks per pass

        def proj_tile(b, t, x_t):
            # fp8 hi+lo DoubleRow: (Wh,xh)+(Wh,xl)+(Wl,xh) passes reproduce
            # the full product of the quantized operands to ~1e-3; weights
            # are host-scaled by WS=64 (fp8 subnormal dodge), folded back in
            # the evacuation scale.
            sb = st[b]
            xh, xl = x_t["h"], x_t["l"]
            for h in range(HPC):
                for wh, wl, dsts, bias, scl in (
                    (wsb["qh"], wsb["ql"], sb["qT_h"], bq_sb, SCALE / WS),
                    (wsb["kh"], wsb["kl"], sb["kT_h"], bk_sb, 1.0 / WS),
                ):
                    ps = psA.tile([128, 512], F32, tag="qkv")
                    hd = slice(h * D, (h + 1) * D)
                    for pi, (w_s, x_s) in enumerate(
                        ((wh, xh), (wh, xl), (wl, xh))
                    ):
                        for ep in range(EP):
                            sl = slice(2 * ep, 2 * ep + 2)
                            nc.tensor.matmul(
                                ps[:, :XT],
                                w_s[:, sl, hd],
                                x_s[:, sl, :],
                                start=(ep == 0 and pi == 0),
                                stop=(ep == EP - 1 and pi == 2),
                                perf_mode=DR,
                            )
                    nc.scalar.activation(
                        dsts[h][:, t * XT:(t + 1) * XT],
                        ps[:, :XT],
                        AF.Identity,
                        bias=bias[:, h:h + 1],
                        scale=scl,
                    )
            for sti in range(XT // 128):
                ps = psA.tile([128, 512], F32, tag="qkv")
                ts = slice(sti * 128, (sti + 1) * 128)
                for pi, (x_s, w_s) in enumerate(
                    ((xh, wsb["vh"]), (xl, wsb["vh"]), (xh, wsb["vl"]))
                ):
                    for ep in range(EP):
                        sl = slice(2 * ep, 2 * ep + 2)
                        nc.tensor.matmul(
                            ps[:, :M],
                            x_s[:, sl, ts],
                            w_s[:, sl, :],
                            start=(ep == 0 and pi == 0),
                            stop=(ep == EP - 1 and pi == 2),
                            perf_mode=DR,
                        )
                nc.vector.tensor_scalar_mul(
                    sb["v_sb"][:, t * (XT // 128) + sti, :], ps[:, :M], 1.0 / WS
                )

        def attn_qtile(b, h, qt):
            """Emit the k-loop (scores/exp with av+Z lagged 2 k-tiles so the
            PE wait-queue never fills on a pending exp) plus the reciprocal;
            returns a closure that emits the chain tail (transpose →
            broadcast → normalize), to be called once later PE work can
            cover its latency."""
            sb = st[b]
            q_rhs = sb["qT_h"][h][:, qt * QT:(qt + 1) * QT]
            ut = psB.tile([128, 512], F32, tag="ut")
            # Z accumulator: chunk qi lives at column qi*4 (16-byte spacing —
            # PSUM matmul outputs at unaligned 4-byte offsets misbehave)
            zt = psZ.tile([128, 512], F32, tag="z")
            nkt = (qt + 1) * (QT // 128)
            pend = []

            def flush(keep):
                while len(pend) > keep:
                    kt0, e0, lo0 = pend.pop(0)
                    nc.tensor.matmul(
                        ut[:, lo0:],
                        sb["v_h"][h][:, kt0, :],
                        e0[:, lo0:],
                        start=(kt0 == 0),
                        stop=(kt0 == nkt - 1),
                    )
                    # Z accumulation: per 128-wide q-chunk, a transposed
                    # tiny matmul (output [128,1], ~free on the PE) sums e
                    # over this k-tile's 128 keys, accumulating in PSUM.
                    # PSUM start/stop are bank-granular (2KB zero-region):
                    # exactly one start on the bank's first write (arms
                    # pending-zero for every byte, so each column's first
                    # write SETs) and one stop on the very last write.
                    for qi in range(lo0 // 128, 4):
                        nc.tensor.matmul(
                            zt[:, qi * 4:qi * 4 + 1],
                            e0[:, qi * 128:(qi + 1) * 128],
                            on_sb[:],
                            start=(kt0 == 0 and qi == 0),
                            stop=(kt0 == nkt - 1 and qi == 3),
                        )

            for kt in range(nkt):
                jj = kt - qt * (QT // 128)
                # columns < jj*128 of this k-tile's block are causally
                # masked; diagonal tiles come last in the k-loop, so
                # accumulating only the valid sub-range is exact.
                lo = max(jj, 0) * 128
                sc = psC.tile([128, 512], F32, tag="sc")
                nc.tensor.matmul(
                    sc[:, lo:],
                    sb["kT_h"][h][:, kt * 128:(kt + 1) * 128],
                    q_rhs[:, lo:],
                    start=True,
                    stop=(jj < 0),
                )
                e = epool.tile([128, 512], F16, tag="e")
                if jj >= 0:
                    # causal mask for the diagonal block: accumulate tb into
                    # the scores bank on the PE itself (tbT^T @ I = tb,
                    # 53ns) — no DVE round-trip in the exp chain
                    nc.tensor.matmul(
                        sc[:, jj * 128:(jj + 1) * 128],
                        tbT_sb[:],
                        id_sb[:],
                        start=False,
                        stop=True,
                    )
                nc.scalar.activation(e[:, lo:], sc[:, lo:], AF.Exp)
                pend.append((kt, e, lo))
                flush(3)
            flush(0)
            rc4 = rp.tile([128, 16], F16, tag="rc4")
            with nc.allow_low_precision(reason="1/Z fp16 rel err ~5e-4"):
                nc.vector.reciprocal(rc4[:], zt[:, 0:16])

            rbbox = {}

            def chain_a():
                # broadcast-transpose 1/Z from q-partition columns back to a
                # row: rc_col^T @ I gives a [1,128] row at partition 0 (fp16
                # matmul, 53ns each).  Scratch lives in a borrowed scores
                # slot so it can't clobber the next head's Z accumulator.
                zr = psC.tile([128, 512], F32, tag="sc")
                for qi in range(4):
                    nc.tensor.matmul(
                        zr[0:1, qi * 128:(qi + 1) * 128],
                        rc4[:, qi * 4:qi * 4 + 1],
                        id_sb[:],
                        start=(qi == 0),
                        stop=(qi == 3),
                    )
                rec_row = rp.tile([1, 512], F32, tag="rec")
                if b == 1:
                    nc.scalar.copy(rec_row[:], zr[0:1, :])
                else:
                    nc.vector.tensor_copy(rec_row[:], zr[0:1, :])
                rb = rp.tile([128, 512], F32, tag="rb")
                nc.gpsimd.partition_broadcast(rb[:], rec_row[:])
                rbbox["rb"] = rb

            def chain_b():
                # emitted after other DVE work so the mult (which waits on
                # the gpsimd broadcast) never head-of-line blocks evacs
                nc.vector.tensor_tensor(
                    sb["oT_h"][h][:, qt * QT:(qt + 1) * QT], ut[:],
                    rbbox["rb"][:], ALU.mult
                )

            return chain_a, chain_b

        def out_proj(b, qt, spread=False, alt_pool=False, rmod=3):
            # spread mode also rotates PSUM over psA+psC (scores banks are
            # free once the last exp ran), lifting the 2-bank throttle
            sb = st[b]
            unit = 0
            for qi4 in range(QT // 128):
                qi = qt * (QT // 128) + qi4
                osb = None
                for et in range(E // ET):
                    if alt_pool and unit % 2 == 1:
                        ps = psC.tile([128, 512], F32, tag="sc")
                    else:
                        ps = psA.tile([128, 512], F32, tag="qkv")
                    for h in range(HPC):
                        nc.tensor.matmul(
                            ps[:],
                            sb["oT_h"][h][:, qi * 128:(qi + 1) * 128],
                            wo_sb[:, h, et * ET:(et + 1) * ET],
                            start=(h == 0),
                            stop=(h == HPC - 1),
                        )
                    if osb is None:
                        osb = osp.tile([128, 2 * ET], F16, tag="osb")
                    half = osb[:, (et % 2) * ET:(et % 2 + 1) * ET]
                    r = unit % (2 if spread else rmod)
                    if r == 1:
                        nc.scalar.copy(half, ps[:])
                    else:
                        nc.vector.tensor_copy(half, ps[:])
                    if et % 2 == 1:
                        nc.sync.dma_start(
                            o[b, qi * 128:(qi + 1) * 128,
                              (et - 1) * ET:(et + 1) * ET],
                            osb[:],
                        )
                        osb = None
                    unit += 1

        def load_x(b, t):
            x_t = {s: xpool.tile([128, EO, XT], F8, tag=f"x{s}", name=f"xt{s}")
                   for s in "hl"}
            for s in "hl":
                nc.sync.dma_start(x_t[s][:], xT[s][b, t])
            return x_t

        # ---- batch 0 projections ----
        new_batch(0)
        for t in range(NT):
            if t == 0:
                x_t = x_first
            elif t == 1:
                x_t = x_second
            else:
                x_t = load_x(0, t)
            proj_tile(0, t, x_t)
            if t == 2:
                # wo is not needed until the first out-projection; keep it
                # off the startup DMA critical path
                nc.scalar.dma_start(
                    wo_sb[:], wo.rearrange("(h p) e -> p h e", p=128))

        # ---- batch 0 attention, interleaved with batch 1 projections ----
        new_batch(1)
        xq = [load_x(1, 0)]  # prefetch queue for b1 x tiles

        for qt in range(NQT):
            a0, b0f = attn_qtile(0, 0, qt)
            a1, b1f = attn_qtile(0, 1, qt)
            a0()
            t = 2 * qt
            if t + 1 < NT:
                xq.append(load_x(1, t + 1))
            proj_tile(1, t, xq.pop(0))
            a1()
            b0f()
            t = 2 * qt + 1
            if t + 1 < NT:
                xq.append(load_x(1, t + 1))
            proj_tile(1, t, xq.pop(0))
            b1f()
            out_proj(0, qt)

        # ---- batch 1 attention, out-projections deferred one q-tile so the
        # softmax chains always have PE work covering their latency ----
        for qt in range(NQT):
            c0 = attn_qtile(1, 0, qt)
            if qt == NQT - 1:
                # last q-tile: emit the previous out-projection between the
                # two head loops so its output DMAs drain before the final
                # window (the DMA device is saturated at the end otherwise)
                out_proj(1, qt - 1, spread=True, rmod=5)
            a0, b0f = c0
            a1, b1f = attn_qtile(1, 1, qt)
            a0()
            a1()
            if 1 <= qt < NQT - 1:
                out_proj(1, qt - 1, rmod=5)
            b0f()
            b1f()
        out_proj(1, NQT - 1, spread=True, alt_pool=True)


_NC_CACHE = None


def _get_nc():
    global _NC_CACHE
    if _NC_CACHE is None:
        _NC_CACHE = build_nc()
    return _NC_CACHE


def _split8(a):
    import ml_dtypes
    F8NP = ml_dtypes.float8_e4m3
    hi = a.astype(F8NP)
    lo = (a - hi.astype(np.float32)).astype(F8NP)
    return hi, lo


def _prep_inputs(x, Wq, bq, Wk, bk, Wv, bv, Wo, bo):
    x = np.asarray(x, dtype=np.float32)
    xT = np.ascontiguousarray(x.transpose(0, 2, 1))          # [B, E, S]
    xh, xl = _split8(xT)
    # pre-tile to the SBUF layout [B, NT, 128, EO, XT] so each x-tile DMA is
    # one 4KB-contiguous run per partition (256B runs pay a 2x DMA penalty)
    def _pretile(a):
        return np.ascontiguousarray(
            a.reshape(B, EO, 128, NT, XT).transpose(0, 3, 2, 1, 4))
    xh, xl = _pretile(xh), _pretile(xl)
    tbT_np = np.where(
        np.arange(128)[:, None] <= np.arange(128)[None, :], 0.0, MASK_BIAS
    ).astype(np.float16).T.copy()
    id_np = np.eye(128, dtype=np.float16)
    in_maps = []
    for c in range(NCORES):
        sl = slice(c * M, (c + 1) * M)
        m = {
            "xh": xh, "xl": xl,
            "wo": np.ascontiguousarray(np.asarray(Wo)[:, sl].T.astype(np.float16)),
        }
        for n, W in (("q", Wq), ("k", Wk), ("v", Wv)):
            Wp = np.ascontiguousarray(
                np.asarray(W)[sl, :].T.astype(np.float32)) * WS
            wh8, wl8 = _split8(Wp)                            # [E, M]
            for s, w8 in (("h", wh8), ("l", wl8)):
                m[f"w{n}{s}"] = np.ascontiguousarray(
                    w8.reshape(EO, 128, M).transpose(1, 0, 2))
        in_maps.append({
            **m,
            "bq": np.ascontiguousarray(
                (np.asarray(bq)[sl].astype(np.float32) * SCALE).reshape(HPC, 128).T
            ),
            "bk": np.ascontiguousarray(
                np.asarray(bk)[sl].astype(np.float32).reshape(HPC, 128).T
            ),
            "tb": tbT_np,
            "ident": id_np,
        })
    return in_maps


def run(inputs, trace=False):
    in_maps = _prep_inputs(
        inputs["x"], inputs["Wq"], inputs["bq"], inputs["Wk"], inputs["bk"],
        inputs["Wv"], inputs["bv"], inputs["Wo"], inputs["bo"],
    )
    nc = _get_nc()
    res = run_bass_kernel_spmd(nc, in_maps, list(range(NCORES)), trace=trace)
    acc = np.zeros((B, S, E), dtype=np.float64)
    for r in res.results:
        acc += r["o"].astype(np.float64)
    acc += np.asarray(inputs["bo"], dtype=np.float64)[None, None, :]
    acc += (np.asarray(inputs["bv"], dtype=np.float64)
            @ np.asarray(inputs["Wo"], dtype=np.float64).T)[None, None, :]
    return acc.astype(np.float32), res


def kernel(**inputs):
    out, _ = run(inputs, trace=False)
    return out
